# revision 47
# baseline (speedup 1.0000x reference)
"""Causal multi-head attention (B=2, T=2048, C=1024, H=16, D=64) on 8 trn2 cores.

Sharding: core c -> (batch c//4, head-group c%4 of 4 heads / 256 channels).
Each core computes q/k/v for its head group, causal attention, and a partial
output projection y_part[2048,1024] = attnout_g @ wo_g.T. The host sums the 4
per-group partials of each batch (the "all-reduce after wo" done host-side).

Device kernel (per core, SPMD identical program), bf16 datapath, SOFTWARE
PIPELINED across iterations: q/k/v/x/wo live in two persistent buffer sets;
iteration n runs attention (C/D) on set p while the projections (B) for
iteration n+1 are emitted as PE gap-filler units into set 1-p, interleaved
one unit per attention chunk. Phase C is exp(ACT)-paced, so the B/D filler
matmuls soak up the PE idle the in-order queue would otherwise leave.

  phase B: q^T,k^T packed 2-heads-per-128-partitions (head h in rows
           64*(h%2)..+64 of slot h//2); scores contract K=64 on those
           slices directly (no zero-padding, no scatter copies). v is
           computed NATURALLY (v[t,o] = xT-slice.T @ wvT) into
           vaug[tk, i, h, 0:64] with a ones column at 64.
  phase C: per head/tq-chunk(512): ST[tk128,tq512] = kT.T @ qT (K=64);
           P = exp(ST/8) on ScalarE -> bf16; causal staircase masked on
           P (cols >= 128*rmin only; the rest is never read). PV is
           FLIPPED: out[tq128, 65] = P-slice.T @ vaug (F=65); the 4
           tq-slice accumulation groups run as 2 passes of 2 (one PSUM
           bank per open group - a group's start clobbers other partials
           in the same bank). Column 64 is the softmax denominator,
           per-PARTITION, so normalization is an approx-recip +
           tensor_scalar_mul. attnout natural [tq, ch] is PE-transposed
           back to attnoutT[ch, t] for the out-projection.
  phase D: y = attnoutT.T @ woT per 128-row tile, emitted as filler units.

PSUM banks: scores 2x[128,2,512] (4) + PV/transpose 2x[128,512] (2) +
B/D shared ring 2x[128,512] (2).
"""
import hashlib
import numpy as np
import ml_dtypes
from collections import deque

import jax
import jax.numpy as jnp
from jax.sharding import Mesh, PartitionSpec
from jax.experimental.shard_map import shard_map

import concourse.bass as bass
import concourse.tile as tile
from concourse import bacc, mybir
from concourse import bass2jax
from concourse.bass2jax import _bass_exec_p, install_neuronx_cc_hook, partition_id_tensor

B, T, C = 2, 2048, 1024
H = 16
D = C // H            # 64
SCALE = D ** -0.5     # 0.125
N_CORES = 8
HG = H // (N_CORES // B)   # heads per core = 4
HC = HG * D                # channels per core = 256
KT = C // 128              # 8 contraction tiles
NT = T // 128              # 16 row tiles
NJ = T // 512              # 4 tq tiles
F32 = mybir.dt.float32
BF16 = mybir.dt.bfloat16

import os
K_STAIR = os.environ.get("K_STAIR", "pool")   # pool affine_select | dve mult
K_YDMA = os.environ.get("K_YDMA", "sp")       # y DMA issue queue
LOOKAHEAD = int(os.environ.get("K_LA", "2"))  # chunks in flight exp->PV
PT_BUFS = int(os.environ.get("K_PTB", "10"))  # pt pool depth (2-pass PV)
K_TRIM = os.environ.get("K_TRIM", "0") == "1"  # F-trim staircase scores/exp


# ---------------------------------------------------------------- device code

def _build_nc(block_info, n_uniq, shifts=(), loop_n=None, phases="BCVD",
              cast_dma=False, dbg=False):
    """block_info[j][i] = None (skip) | -1 (full) | ("st", slot) (causal
    staircase) | ("mk", idx>=0) (general mask tile multiplied on DVE)."""
    n_shift = len(shifts)
    # per stair slot: first tq-128-slice that reads this tile
    rmin = [max(0, -(-(s - 127) // 128)) for s in shifts]
    use_stm = K_STAIR == "dve" and n_shift > 0
    nc = bacc.Bacc("TRN2", target_bir_lowering=False, debug=False,
                   num_devices=N_CORES)
    xT_ap = nc.dram_tensor("xT", [C, T], BF16, kind="ExternalInput").ap()
    # weights host-packed so each SBUF partition's bytes are contiguous in
    # DRAM (large descriptors; scattered 1KB descriptors measured ~4x slower)
    wqT_ap = nc.dram_tensor("wqT", [128, KT * HC], BF16, kind="ExternalInput").ap()
    wkT_ap = nc.dram_tensor("wkT", [128, KT * HC], BF16, kind="ExternalInput").ap()
    wvT_ap = nc.dram_tensor("wvT", [128, KT * HC], BF16, kind="ExternalInput").ap()
    woT_ap = nc.dram_tensor("woT", [128, 2 * C], BF16, kind="ExternalInput").ap()
    if n_uniq > 0:
        mk_ap = nc.dram_tensor("mk", [128, n_uniq * 512], F32,
                               kind="ExternalInput").ap()
    if use_stm:
        stm_ap = nc.dram_tensor("stm", [128, n_shift * 512], BF16,
                                kind="ExternalInput").ap()
    y_ap = nc.dram_tensor("y", [T, C], BF16, kind="ExternalOutput").ap()
    if dbg:
        dqT_ap = nc.dram_tensor("dqT", [128, 2 * T], BF16, kind="ExternalOutput").ap()
        dkT_ap = nc.dram_tensor("dkT", [128, 2 * T], BF16, kind="ExternalOutput").ap()
        dva_ap = nc.dram_tensor("dva", [128, NT * HG * 66], BF16, kind="ExternalOutput").ap()
        dao_ap = nc.dram_tensor("dao", [128, 2 * T], BF16, kind="ExternalOutput").ap()

    with tile.TileContext(nc) as tc:
        with (
            nc.allow_low_precision(reason="bf16 matmul pipeline"),
            tc.tile_pool(name="glob", bufs=1) as pg,
            tc.tile_pool(name="warm", bufs=1) as pwarm,
            tc.tile_pool(name="wp", bufs=6) as pwp,
            tc.tile_pool(name="ao", bufs=2) as pao,
            tc.tile_pool(name="an", bufs=2) as pan,
            tc.tile_pool(name="mks", bufs=2) as pmks,
            tc.tile_pool(name="pt", bufs=PT_BUFS) as ppt,
            tc.tile_pool(name="small", bufs=8) as psm,
            tc.tile_pool(name="ys", bufs=3) as pys,
            # PSUM: one open accumulation group per 2KB bank (a group's
            # start clobbers other partials sharing its bank).
            tc.tile_pool(name="pst", bufs=2, space="PSUM") as pst,   # 4 banks
            tc.tile_pool(name="ppv", bufs=2, space="PSUM") as ppv,   # 2 banks
            tc.tile_pool(name="pbd", bufs=2, space="PSUM") as pbd,   # 2 banks
        ):
            ident = pg.tile([128, 128], F32)       # PE-transpose identity
            if use_stm:
                stm = pg.tile([128, n_shift, 512], BF16)
                nc.sync.dma_start(
                    stm[:], stm_ap.rearrange("p (s f) -> p s f", f=512))

            # warm the Exp table while DMAs run
            wtile = pwarm.tile([1, 16], F32)
            nc.vector.memset(wtile[:], 0.0)
            nc.scalar.activation(wtile[:], wtile[:],
                                 mybir.ActivationFunctionType.Exp)
            import concourse.masks as _masks
            _masks.make_identity(nc, ident[:])

            # persistent double-buffered projection sets (x single-buffered:
            # its DMA for body n+1 waits body n's last projection read)
            xTs = [pg.tile([128, KT, T], BF16, name="xTs")] * 2
            qTs = [pg.tile([128, 2, T], BF16, name=f"qTs{s}") for s in range(2)]
            kTs = [pg.tile([128, 2, T], BF16, name=f"kTs{s}") for s in range(2)]
            vas = [pg.tile([128, NT, HG, 66], BF16, name=f"vas{s}") for s in range(2)]
            wos = [pg.tile([128, 2, C], BF16, name=f"wos{s}") for s in range(2)]

            def dma_set(s, it):
                """Issue input DMAs for projection set s."""
                xT = xTs[s]
                xr = xT_ap.rearrange("(k p) t -> k p t", p=128)
                w_ts = [pwp.tile([128, KT, HC], BF16, tag="w",
                                 name=f"w{qk}_{it}") for qk in range(3)]
                wqr = wqT_ap.rearrange("p (k m) -> p k m", k=KT)
                nc.sync.dma_start(w_ts[0][:, 0:KT // 2, :], wqr[:, 0:KT // 2, :])
                nc.sync.dma_start(xT[:, 0, :], xr[0])
                nc.sync.dma_start(w_ts[0][:, KT // 2:, :], wqr[:, KT // 2:, :])
                nc.sync.dma_start(xT[:, 1, :], xr[1])
                nc.sync.dma_start(w_ts[1][:], wkT_ap.rearrange("p (k m) -> p k m", k=KT))
                for kc in range(2, KT):
                    nc.sync.dma_start(xT[:, kc, :], xr[kc])
                nc.sync.dma_start(w_ts[2][:], wvT_ap.rearrange("p (k m) -> p k m", k=KT))
                nc.sync.dma_start(wos[s][:], woT_ap.rearrange("p (k m) -> p k m", k=2))
                return w_ts

            def b_units(s, w_ts, it):
                """Projection work for set s as a list of filler closures."""
                units = []
                if "B" not in phases:
                    return units
                xT, qT, kTp, vaug = xTs[s], qTs[s], kTs[s], vas[s]
                for qk in range(2):
                    dst = qT if qk == 0 else kTp
                    for m in range(2):
                        for j in range(4):
                            def u(qk=qk, m=m, j=j, dst=dst):
                                pss = pbd.tile([128, 512], F32, tag="bd",
                                               name=f"qk{qk}_{m}_{j}_{it}")
                                for kc in range(KT):
                                    nc.tensor.matmul(
                                        pss[:],
                                        w_ts[qk][:, kc, 128 * m:128 * (m + 1)],
                                        xT[:, kc, 512 * j:512 * (j + 1)],
                                        start=(kc == 0), stop=(kc == KT - 1))
                                nc.vector.tensor_copy(
                                    dst[:, m, 512 * j:512 * (j + 1)], pss[:])
                            units.append(u)
                for i in range(NT):
                    def u(i=i):
                        pvn = pbd.tile([128, 512], F32, tag="bd",
                                       name=f"vn{i}_{it}")
                        flat = pvn[:, 0:HC]
                        for kc in range(KT):
                            nc.tensor.matmul(
                                flat, xT[:, kc, 128 * i:128 * (i + 1)],
                                w_ts[2][:, kc, :],
                                start=(kc == 0), stop=(kc == KT - 1))
                        nc.vector.tensor_copy(
                            vaug[:, i, :, 0:D],
                            flat.rearrange("p (h d) -> p h d", h=HG))
                    units.append(u)
                units.append(lambda: nc.vector.memset(vaug[:, :, :, D], 1.0))
                return units

            def body(it, rd, wr):
                """Attention on set rd; fillers project into set wr."""
                fillers = deque()
                if wr is not None:
                    w_ts = dma_set(wr, it)
                if n_uniq > 0:
                    mks = pmks.tile([128, n_uniq, 512], F32, tag="mks",
                                    name=f"mks_{it}")
                    nc.sync.dma_start(mks[:], mk_ap.rearrange("p (u f) -> p u f", f=512))
                deferred = []
                if wr is not None:
                    if wr == rd:
                        # non-pipelined: projections must run after attention
                        deferred = b_units(wr, w_ts, it)
                    else:
                        fillers.extend(b_units(wr, w_ts, it))
                if "C" not in phases:
                    for u in (*fillers, *deferred):
                        u()
                    return
                qT, kTp, vaug = qTs[rd], kTs[rd], vas[rd]
                woT = wos[rd]
                attnoutT = pao.tile([128, 2, T], BF16, tag="ao", name=f"ao_{it}")

                def d_unit(j, tp, tsub):
                    def emit():
                        t = 4 * j + 2 * tp + tsub
                        ys = d_unit.ys
                        if tsub == 0:
                            ys = d_unit.ys = pys.tile(
                                [128, 2, C], BF16, tag="ys", name=f"ys{t}_{it}")
                        for o2 in range(2):
                            yps = pbd.tile([128, 512], F32, tag="bd",
                                           name=f"yps{t}_{o2}_{it}")
                            for kc in range(2):
                                nc.tensor.matmul(
                                    yps[:],
                                    attnoutT[:, kc, 128 * t:128 * (t + 1)],
                                    woT[:, kc, 512 * o2:512 * (o2 + 1)],
                                    start=(kc == 0), stop=(kc == 1))
                            nc.vector.tensor_copy(
                                ys[:, tsub, 512 * o2:512 * (o2 + 1)], yps[:])
                        if tsub == 1:
                            r0 = 512 * j + 256 * tp
                            eng = {"pool": nc.gpsimd, "act": nc.scalar,
                                   "sp": nc.sync}[K_YDMA]
                            eng.dma_start(
                                y_ap[r0:r0 + 256, :].rearrange(
                                    "(tt p) o -> p tt o", p=128),
                                ys[:])
                    return emit
                d_unit.ys = None

                for j in range(NJ):
                    blocks = [(i, bi) for i, bi in enumerate(block_info[j])
                              if bi is not None]
                    chunks = [blocks[c:c + 2] for c in range(0, len(blocks), 2)]
                    # PV bookkeeping: contributors per tq-slice
                    contrib = [[] for _ in range(4)]
                    for i, bi in blocks:
                        lo = rmin[bi[1]] if (isinstance(bi, tuple)
                                             and bi[0] == "st") else 0
                        for rp in range(lo, 4):
                            contrib[rp].append(i)
                    anat = pan.tile([128, 2, 4, 2, D], F32, tag="an",
                                    name=f"an{j}_{it}")
                    for h in range(HG):
                        m, hh = h // 2, h % 2
                        r0 = 64 * hh
                        jsl = slice(512 * j, 512 * (j + 1))
                        seen = [0] * 4
                        pvs = [None] * 4

                        def emit_pv(pt, ch, rps):
                            if "V" not in phases:
                                return
                            for c, (i, bi) in enumerate(ch):
                                lo = rmin[bi[1]] if (isinstance(bi, tuple)
                                                     and bi[0] == "st") else 0
                                for rp in rps:
                                    if rp < lo:
                                        continue
                                    seen[rp] += 1
                                    nc.tensor.matmul(
                                        pvs[rp][:, 0:65],
                                        pt[:, c, 128 * rp:128 * (rp + 1)],
                                        vaug[:, i, h, 0:65],
                                        start=(seen[rp] == 1),
                                        stop=(seen[rp] == len(contrib[rp])))

                        def norm(rp):
                            if not contrib[rp]:
                                nc.vector.memset(anat[:, m, rp, hh, :], 0.0)
                                return
                            dn = psm.tile([128, 1], F32, tag="dn")
                            nc.vector.tensor_copy(dn[:], pvs[rp][:, D:D + 1])
                            rc = psm.tile([128, 1], F32, tag="rc")
                            nc.vector.reciprocal_approx_fast(rc[:], dn[:])
                            nc.vector.tensor_scalar_mul(
                                anat[:, m, rp, hh, :], pvs[rp][:, 0:D], rc[:])

                        # pass 1: tq-slices 0,1 accumulate while chunks flow
                        if "V" in phases:
                            for rp in (0, 1):
                                if contrib[rp]:
                                    pvs[rp] = ppv.tile(
                                        [128, 512], F32, tag="pv",
                                        name=f"pv{h}_{j}_{rp}_{it}")
                        pend = deque()
                        pts = []
                        for ch in chunks:
                            nsub = len(ch)
                            # staircase blocks: cols < 128*rmin are never
                            # read downstream - trim the score matmul and exp
                            cut = [128 * rmin[bi[1]]
                                   if K_TRIM and isinstance(bi, tuple)
                                   and bi[0] == "st"
                                   else 0 for _, bi in ch]
                            st = pst.tile([128, 2, 512], F32, tag="st",
                                          name=f"st{h}_{j}_{it}")
                            for c, (i, bi) in enumerate(ch):
                                nc.tensor.matmul(
                                    st[:, c, cut[c]:],
                                    kTp[r0:r0 + 64, m, 128 * i:128 * (i + 1)],
                                    qT[r0:r0 + 64, m,
                                       512 * j + cut[c]:512 * (j + 1)],
                                    start=True, stop=True)
                            pt = ppt.tile([128, 2, 512], BF16, tag="pt")
                            # one exp per chunk when untrimmed: fixed per-op
                            # ACT cost amortizes over 1024 elements
                            if not any(cut):
                                nc.scalar.activation(
                                    pt[:, 0:nsub, :], st[:, 0:nsub, :],
                                    mybir.ActivationFunctionType.Exp,
                                    scale=SCALE)
                            else:
                                for c in range(nsub):
                                    nc.scalar.activation(
                                        pt[:, c, cut[c]:], st[:, c, cut[c]:],
                                        mybir.ActivationFunctionType.Exp,
                                        scale=SCALE)
                            for c, (i, bi) in enumerate(ch):
                                if not isinstance(bi, tuple):
                                    continue
                                if bi[0] == "st":
                                    # zero exp output where p > f - s
                                    # (cols < 128*rmin are never read)
                                    s = shifts[bi[1]]
                                    c0 = 128 * rmin[bi[1]]
                                    if use_stm:
                                        nc.vector.tensor_mul(
                                            pt[:, c, c0:], pt[:, c, c0:],
                                            stm[:, bi[1], c0:])
                                    else:
                                        nc.gpsimd.affine_select(
                                            out=pt[:, c, c0:],
                                            in_=pt[:, c, c0:],
                                            compare_op=mybir.AluOpType.is_ge,
                                            fill=0.0,
                                            base=c0 - s,
                                            pattern=[[1, 512 - c0]],
                                            channel_multiplier=-1)
                            pend.append((pt, ch))
                            pts.append((pt, ch))
                            if len(pend) > LOOKAHEAD:
                                emit_pv(*pend.popleft(), (0, 1))
                            if fillers:
                                fillers.popleft()()
                        while pend:
                            emit_pv(*pend.popleft(), (0, 1))
                        if "V" not in phases:
                            continue
                        norm(0)
                        norm(1)
                        # pass 2: tq-slices 2,3 re-sweep the kept pt tiles
                        for rp in (2, 3):
                            if contrib[rp]:
                                pvs[rp] = ppv.tile(
                                    [128, 512], F32, tag="pv",
                                    name=f"pv{h}_{j}_{rp}_{it}")
                        for pt, ch in pts:
                            emit_pv(pt, ch, (2, 3))
                        norm(2)
                        norm(3)

                        if hh == 1:
                            # both heads of pair m done: transpose natural
                            # attnout back to channel-major for out-proj
                            for half in range(2):
                                trp = ppv.tile([128, 512], F32, tag="pv",
                                               name=f"tr{m}_{half}_{j}_{it}")
                                for q2 in range(2):
                                    rp = 2 * half + q2
                                    nc.tensor.transpose(
                                        trp[:, 128 * q2:128 * (q2 + 1)],
                                        anat[:, m, rp, :, :].rearrange(
                                            "p a b -> p (a b)"),
                                        ident[:])
                                a0 = 512 * j + 256 * half
                                nc.vector.tensor_copy(
                                    attnoutT[:, m, a0:a0 + 256],
                                    trp[:, 0:256])

                    if "D" in phases and "V" in phases:
                        for tp in range(2):
                            for tsub in range(2):
                                fillers.append(d_unit(j, tp, tsub))
                while fillers:
                    fillers.popleft()()
                for u in deferred:
                    u()
                if dbg:
                    nc.sync.dma_start(dqT_ap.rearrange("p (a t) -> p a t", a=2), qT[:])
                    nc.sync.dma_start(dkT_ap.rearrange("p (a t) -> p a t", a=2), kTp[:])
                    nc.sync.dma_start(dva_ap.rearrange("p (i h e) -> p i h e", i=NT, h=HG), vaug[:])
                    nc.sync.dma_start(dao_ap.rearrange("p (a t) -> p a t", a=2), attnoutT[:])

            def prologue():
                w_ts = dma_set(0, "p")
                for u in b_units(0, w_ts, "p"):
                    u()

            prologue()
            if loop_n is None:
                body(0, 0, None)
            elif loop_n % 2:
                # odd loop count: non-pipelined fallback, single set
                with tc.For_i(0, loop_n, 1, staggered_reset=True):
                    body(0, 0, 0)
            else:
                unroll = 4 if loop_n % 4 == 0 else 2
                with tc.For_i(0, loop_n // unroll, 1, staggered_reset=True):
                    for it in range(unroll):
                        body(it, it % 2, 1 - it % 2)

    nc.compile()
    return nc


# ---------------------------------------------------------------- run harness

def _install_verbose_hook():
    install_neuronx_cc_hook()
    try:
        import libneuronxla
    except ImportError:
        return
    import traceback
    inner = bass2jax.neuronx_cc_hook

    def wrapped(*a, **kw):
        try:
            return inner(*a, **kw)
        except BaseException:
            traceback.print_exc()
            raise
    libneuronxla.neuronx_cc = wrapped


class _SpmdRunner:
    def __init__(self, nc, n_cores):
        _install_verbose_hook()
        self.nc, self.n_cores = nc, n_cores
        pname = nc.partition_id_tensor.name if nc.partition_id_tensor else None
        in_names, out_names, out_avals = [], [], []
        for alloc in nc.m.functions[0].allocations:
            if not isinstance(alloc, mybir.MemoryLocationSet):
                continue
            name = alloc.memorylocations[0].name
            if alloc.kind == "ExternalInput":
                if name != pname:
                    in_names.append(name)
            elif alloc.kind == "ExternalOutput":
                out_names.append(name)
                out_avals.append(jax.core.ShapedArray(
                    tuple(alloc.tensor_shape), mybir.dt.np(alloc.dtype)))
        self.in_names, self.out_names, self.out_avals = in_names, out_names, out_avals
        n_params = len(in_names)
        all_in = list(in_names) + list(out_names)
        if pname is not None:
            all_in.append(pname)

        def _body(*args):
            operands = list(args)
            if pname is not None:
                operands.append(partition_id_tensor())
            return tuple(_bass_exec_p.bind(
                *operands,
                out_avals=tuple(out_avals), in_names=tuple(all_in),
                out_names=tuple(out_names), lowering_input_output_aliases=(),
                sim_require_finite=True, sim_require_nnan=True, nc=nc))

        devices = jax.devices()[:n_cores]
        self.mesh = Mesh(np.asarray(devices), ("core",))
        in_specs = (PartitionSpec("core"),) * (n_params + len(out_names))
        out_specs = (PartitionSpec("core"),) * len(out_names)
        self.fn = jax.jit(shard_map(_body, mesh=self.mesh, in_specs=in_specs,
                                    out_specs=out_specs, check_rep=False),
                          keep_unused=True)
        self._shard = jax.sharding.NamedSharding(self.mesh, PartitionSpec("core"))

    def put_inputs(self, in_maps):
        arrs = []
        for name in self.in_names:
            cat = np.concatenate([np.asarray(m[name]) for m in in_maps], axis=0)
            arrs.append(jax.device_put(cat, self._shard))
        for av in self.out_avals:
            z = np.zeros((self.n_cores * av.shape[0], *av.shape[1:]), av.dtype)
            arrs.append(jax.device_put(z, self._shard))
        return arrs

    def run(self, dev_args):
        outs = self.fn(*dev_args)
        jax.block_until_ready(outs)
        return outs

    def results(self, outs):
        per_core = []
        for c in range(self.n_cores):
            per_core.append({
                name: np.asarray(outs[i]).reshape(
                    self.n_cores, *self.out_avals[i].shape)[c]
                for i, name in enumerate(self.out_names)})
        return per_core


# ---------------------------------------------------------------- host side

def _mask_blocks(mask):
    """Classify transposed 128x512 blocks of the [T,T] mask.

    Returns (block_info, uniq, shifts) where block_info[j][i] is None (all
    masked), -1 (all valid), ("st", slot) (causal staircase valid = p <=
    f - shifts[slot]), or ("mk", idx) (arbitrary pattern from uniq[idx])."""
    m2 = np.asarray(mask).reshape(T, T)
    valid = (m2 != -np.inf)          # [tq, tk]
    validT = valid.T                 # [tk, tq]
    uniq, keys = [], {}
    shifts, shift_keys = [], {}
    p_idx = np.arange(128)[:, None]
    f_idx = np.arange(512)[None, :]
    block_info = []
    for j in range(NJ):
        row = []
        for i in range(NT):
            blk = validT[128 * i:128 * (i + 1), 512 * j:512 * (j + 1)]
            if not blk.any():
                row.append(None)
                continue
            if blk.all():
                row.append(-1)
                continue
            s = 128 * i - 512 * j
            if -512 < s < 512 and np.array_equal(blk, p_idx <= f_idx - s):
                if s not in shift_keys:
                    shift_keys[s] = len(shifts)
                    shifts.append(s)
                row.append(("st", shift_keys[s]))
                continue
            k = hashlib.sha1(np.ascontiguousarray(blk)).hexdigest()
            if k not in keys:
                keys[k] = len(uniq)
                uniq.append(blk.astype(np.float32))
            row.append(("mk", keys[k]))
        block_info.append(row)
    return block_info, uniq, shifts


_CACHE = {}


def _get_runner(block_info, n_uniq, shifts=(), loop_n=None, phases="BCVD",
                cast_dma=True):
    key = (str(block_info), n_uniq, tuple(shifts), loop_n, phases,
           K_STAIR, K_YDMA, LOOKAHEAD, PT_BUFS, K_TRIM)
    if key not in _CACHE:
        nc = _build_nc(block_info, n_uniq, shifts=shifts, loop_n=loop_n,
                       phases=phases, cast_dma=cast_dma)
        _CACHE[key] = _SpmdRunner(nc, N_CORES)
    return _CACHE[key]


def _bf16(a):
    return np.ascontiguousarray(np.asarray(a, np.float32)).astype(
        ml_dtypes.bfloat16)


def _pack_rows(a):
    """[R*128, F] -> [128, R*F]: partition-contiguous packing for fast DMA."""
    r = a.shape[0] // 128
    return np.ascontiguousarray(
        a.reshape(r, 128, a.shape[1]).transpose(1, 0, 2).reshape(128, -1))


def _make_in_maps(x, mask, wq, wk, wv, wo):
    block_info, uniq, shifts = _mask_blocks(mask)
    x = np.asarray(x, np.float32)
    extra = {}
    if uniq:
        mk = np.stack(uniq)    # [u,128,512] -> [128, u*512]
        extra["mk"] = np.ascontiguousarray(
            mk.transpose(1, 0, 2).reshape(128, -1))
    if shifts and K_STAIR == "dve":
        p_idx = np.arange(128)[:, None]
        f_idx = np.arange(512)[None, :]
        stm = np.stack([(p_idx <= f_idx - s).astype(np.float32)
                        for s in shifts])          # [s,128,512]
        extra["stm"] = _bf16(np.ascontiguousarray(
            stm.transpose(1, 0, 2).reshape(128, -1)))
    in_maps = []
    for c in range(N_CORES):
        b, g = c // 4, c % 4
        sl = slice(HC * g, HC * (g + 1))
        in_maps.append({
            "xT": _bf16(x[b].T),
            "wqT": _pack_rows(_bf16(np.asarray(wq)[sl, :].T)),
            "wkT": _pack_rows(_bf16(np.asarray(wk)[sl, :].T)),
            "wvT": _pack_rows(_bf16(np.asarray(wv)[sl, :].T)),
            "woT": _pack_rows(_bf16(np.asarray(wo)[:, sl].T)),
            **extra,
        })
    return in_maps, block_info, len(uniq), tuple(shifts)


def kernel(x, mask, wq, wk, wv, wo):
    in_maps, block_info, n_uniq, shifts = _make_in_maps(x, mask, wq, wk, wv, wo)
    runner = _get_runner(block_info, n_uniq, shifts)
    dev = runner.put_inputs(in_maps)
    res = runner.results(runner.run(dev))
    out = np.zeros((B, T, C), np.float32)
    for c in range(N_CORES):
        out[c // 4] += res[c]["y"].astype(np.float32)
    return out


# revision 51
# speedup vs baseline: 1.0185x; 1.0185x over previous
"""Causal multi-head attention (B=2, T=2048, C=1024, H=16, D=64) on 8 trn2 cores.

Sharding: core c -> (batch c//4, head-group c%4 of 4 heads / 256 channels).
Each core computes q/k/v for its head group, causal attention, and a partial
output projection y_part[2048,1024] = attnout_g @ wo_g.T. The host sums the 4
per-group partials of each batch (the "all-reduce after wo" done host-side).

Device kernel (per core, SPMD identical program), bf16 datapath, SOFTWARE
PIPELINED across iterations: q/k/v/x/wo live in two persistent buffer sets;
iteration n runs attention (C/D) on set p while the projections (B) for
iteration n+1 are emitted as PE gap-filler units into set 1-p, interleaved
one unit per attention chunk. Phase C is exp(ACT)-paced, so the B/D filler
matmuls soak up the PE idle the in-order queue would otherwise leave.

  phase B: q^T,k^T packed 2-heads-per-128-partitions (head h in rows
           64*(h%2)..+64 of slot h//2); scores contract K=64 on those
           slices directly (no zero-padding, no scatter copies). v is
           computed NATURALLY (v[t,o] = xT-slice.T @ wvT) into
           vaug[tk, i, h, 0:64] with a ones column at 64.
  phase C: per head/tq-chunk(512): ST[tk128,tq512] = kT.T @ qT (K=64);
           P = exp(ST/8) on ScalarE -> bf16; causal staircase masked on
           P (cols >= 128*rmin only; the rest is never read). PV is
           FLIPPED: out[tq128, 65] = P-slice.T @ vaug (F=65); the 4
           tq-slice accumulation groups run as 2 passes of 2 (one PSUM
           bank per open group - a group's start clobbers other partials
           in the same bank). Column 64 is the softmax denominator,
           per-PARTITION, so normalization is an approx-recip +
           tensor_scalar_mul. attnout natural [tq, ch] is PE-transposed
           back to attnoutT[ch, t] for the out-projection.
  phase D: y = attnoutT.T @ woT per 128-row tile, emitted as filler units.

PSUM banks: scores 2x[128,2,512] (4) + PV/transpose 2x[128,512] (2) +
B/D shared ring 2x[128,512] (2).
"""
import hashlib
import numpy as np
import ml_dtypes
from collections import deque

import jax
import jax.numpy as jnp
from jax.sharding import Mesh, PartitionSpec
from jax.experimental.shard_map import shard_map

import concourse.bass as bass
import concourse.tile as tile
from concourse import bacc, mybir
from concourse import bass2jax
from concourse.bass2jax import _bass_exec_p, install_neuronx_cc_hook, partition_id_tensor

B, T, C = 2, 2048, 1024
H = 16
D = C // H            # 64
SCALE = D ** -0.5     # 0.125
N_CORES = 8
HG = H // (N_CORES // B)   # heads per core = 4
HC = HG * D                # channels per core = 256
KT = C // 128              # 8 contraction tiles
NT = T // 128              # 16 row tiles
NJ = T // 512              # 4 tq tiles
F32 = mybir.dt.float32
BF16 = mybir.dt.bfloat16

import os
K_STAIR = os.environ.get("K_STAIR", "pool")   # pool affine_select | dve mult
K_YDMA = os.environ.get("K_YDMA", "sp")       # y DMA issue queue
LOOKAHEAD = int(os.environ.get("K_LA", "2"))  # chunks in flight exp->PV
PT_BUFS = int(os.environ.get("K_PTB", "10"))  # pt pool depth (2-pass PV)
K_TRIM = os.environ.get("K_TRIM", "0") == "1"  # F-trim staircase scores/exp
K_DPS = os.environ.get("K_DPS", "0") == "1"    # y psums via pst ring (not pbd)


# ---------------------------------------------------------------- device code

def _build_nc(block_info, n_uniq, shifts=(), loop_n=None, phases="BCVD",
              cast_dma=False, dbg=False):
    """block_info[j][i] = None (skip) | -1 (full) | ("st", slot) (causal
    staircase) | ("mk", idx>=0) (general mask tile multiplied on DVE)."""
    n_shift = len(shifts)
    # per stair slot: first tq-128-slice that reads this tile
    rmin = [max(0, -(-(s - 127) // 128)) for s in shifts]
    use_stm = K_STAIR == "dve" and n_shift > 0
    nc = bacc.Bacc("TRN2", target_bir_lowering=False, debug=False,
                   num_devices=N_CORES)
    xT_ap = nc.dram_tensor("xT", [C, T], BF16, kind="ExternalInput").ap()
    # weights host-packed so each SBUF partition's bytes are contiguous in
    # DRAM (large descriptors; scattered 1KB descriptors measured ~4x slower)
    wqT_ap = nc.dram_tensor("wqT", [128, KT * HC], BF16, kind="ExternalInput").ap()
    wkT_ap = nc.dram_tensor("wkT", [128, KT * HC], BF16, kind="ExternalInput").ap()
    wvT_ap = nc.dram_tensor("wvT", [128, KT * HC], BF16, kind="ExternalInput").ap()
    woT_ap = nc.dram_tensor("woT", [128, 2 * C], BF16, kind="ExternalInput").ap()
    if n_uniq > 0:
        mk_ap = nc.dram_tensor("mk", [128, n_uniq * 512], F32,
                               kind="ExternalInput").ap()
    if use_stm:
        stm_ap = nc.dram_tensor("stm", [128, n_shift * 512], BF16,
                                kind="ExternalInput").ap()
    y_ap = nc.dram_tensor("y", [T, C], BF16, kind="ExternalOutput").ap()
    if dbg:
        dqT_ap = nc.dram_tensor("dqT", [128, 2 * T], BF16, kind="ExternalOutput").ap()
        dkT_ap = nc.dram_tensor("dkT", [128, 2 * T], BF16, kind="ExternalOutput").ap()
        dva_ap = nc.dram_tensor("dva", [128, NT * HG * 66], BF16, kind="ExternalOutput").ap()
        dao_ap = nc.dram_tensor("dao", [128, 2 * T], BF16, kind="ExternalOutput").ap()

    with tile.TileContext(nc) as tc:
        with (
            nc.allow_low_precision(reason="bf16 matmul pipeline"),
            tc.tile_pool(name="glob", bufs=1) as pg,
            tc.tile_pool(name="warm", bufs=1) as pwarm,
            tc.tile_pool(name="wp", bufs=6) as pwp,
            tc.tile_pool(name="ao", bufs=2) as pao,
            tc.tile_pool(name="an", bufs=2) as pan,
            tc.tile_pool(name="mks", bufs=2) as pmks,
            tc.tile_pool(name="pt", bufs=PT_BUFS) as ppt,
            tc.tile_pool(name="small", bufs=8) as psm,
            tc.tile_pool(name="ys", bufs=3) as pys,
            # PSUM: one open accumulation group per 2KB bank (a group's
            # start clobbers other partials sharing its bank).
            tc.tile_pool(name="pst", bufs=2, space="PSUM") as pst,   # 4 banks
            tc.tile_pool(name="ppv", bufs=2, space="PSUM") as ppv,   # 2 banks
            tc.tile_pool(name="pbd", bufs=2, space="PSUM") as pbd,   # 2 banks
        ):
            ident = pg.tile([128, 128], F32)       # PE-transpose identity
            if use_stm:
                stm = pg.tile([128, n_shift, 512], BF16)
                nc.sync.dma_start(
                    stm[:], stm_ap.rearrange("p (s f) -> p s f", f=512))

            # warm the Exp table while DMAs run
            wtile = pwarm.tile([1, 16], F32)
            nc.vector.memset(wtile[:], 0.0)
            nc.scalar.activation(wtile[:], wtile[:],
                                 mybir.ActivationFunctionType.Exp)
            import concourse.masks as _masks
            _masks.make_identity(nc, ident[:])

            # persistent double-buffered projection sets (x single-buffered:
            # its DMA for body n+1 waits body n's last projection read)
            xTs = [pg.tile([128, KT, T], BF16, name="xTs")] * 2
            qTs = [pg.tile([128, 2, T], BF16, name=f"qTs{s}") for s in range(2)]
            kTs = [pg.tile([128, 2, T], BF16, name=f"kTs{s}") for s in range(2)]
            vas = [pg.tile([128, NT, HG, 66], BF16, name=f"vas{s}") for s in range(2)]
            wos = [pg.tile([128, 2, C], BF16, name=f"wos{s}") for s in range(2)]

            def dma_set(s, it):
                """Issue input DMAs for projection set s."""
                xT = xTs[s]
                xr = xT_ap.rearrange("(k p) t -> k p t", p=128)
                w_ts = [pwp.tile([128, KT, HC], BF16, tag="w",
                                 name=f"w{qk}_{it}") for qk in range(3)]
                wqr = wqT_ap.rearrange("p (k m) -> p k m", k=KT)
                nc.sync.dma_start(w_ts[0][:, 0:KT // 2, :], wqr[:, 0:KT // 2, :])
                nc.sync.dma_start(xT[:, 0, :], xr[0])
                nc.sync.dma_start(w_ts[0][:, KT // 2:, :], wqr[:, KT // 2:, :])
                nc.sync.dma_start(xT[:, 1, :], xr[1])
                nc.sync.dma_start(w_ts[1][:], wkT_ap.rearrange("p (k m) -> p k m", k=KT))
                for kc in range(2, KT):
                    nc.sync.dma_start(xT[:, kc, :], xr[kc])
                nc.sync.dma_start(w_ts[2][:], wvT_ap.rearrange("p (k m) -> p k m", k=KT))
                nc.sync.dma_start(wos[s][:], woT_ap.rearrange("p (k m) -> p k m", k=2))
                return w_ts

            def b_units(s, w_ts, it):
                """Projection work for set s as a list of filler closures."""
                units = []
                if "B" not in phases:
                    return units
                xT, qT, kTp, vaug = xTs[s], qTs[s], kTs[s], vas[s]
                for qk in range(2):
                    dst = qT if qk == 0 else kTp
                    for m in range(2):
                        for j in range(4):
                            def u(qk=qk, m=m, j=j, dst=dst):
                                pss = pbd.tile([128, 512], F32, tag="bd",
                                               name=f"qk{qk}_{m}_{j}_{it}")
                                for kc in range(KT):
                                    nc.tensor.matmul(
                                        pss[:],
                                        w_ts[qk][:, kc, 128 * m:128 * (m + 1)],
                                        xT[:, kc, 512 * j:512 * (j + 1)],
                                        start=(kc == 0), stop=(kc == KT - 1))
                                nc.vector.tensor_copy(
                                    dst[:, m, 512 * j:512 * (j + 1)], pss[:])
                            units.append(u)
                for i in range(NT):
                    def u(i=i):
                        pvn = pbd.tile([128, 512], F32, tag="bd",
                                       name=f"vn{i}_{it}")
                        flat = pvn[:, 0:HC]
                        for kc in range(KT):
                            nc.tensor.matmul(
                                flat, xT[:, kc, 128 * i:128 * (i + 1)],
                                w_ts[2][:, kc, :],
                                start=(kc == 0), stop=(kc == KT - 1))
                        nc.vector.tensor_copy(
                            vaug[:, i, :, 0:D],
                            flat.rearrange("p (h d) -> p h d", h=HG))
                    units.append(u)
                units.append(lambda: nc.vector.memset(vaug[:, :, :, D], 1.0))
                return units

            def body(it, rd, wr):
                """Attention on set rd; fillers project into set wr."""
                fillers = deque()
                if wr is not None:
                    w_ts = dma_set(wr, it)
                if n_uniq > 0:
                    mks = pmks.tile([128, n_uniq, 512], F32, tag="mks",
                                    name=f"mks_{it}")
                    nc.sync.dma_start(mks[:], mk_ap.rearrange("p (u f) -> p u f", f=512))
                deferred = []
                if wr is not None:
                    if wr == rd:
                        # non-pipelined: projections must run after attention
                        deferred = b_units(wr, w_ts, it)
                    else:
                        fillers.extend(b_units(wr, w_ts, it))
                if "C" not in phases:
                    for u in (*fillers, *deferred):
                        u()
                    return
                qT, kTp, vaug = qTs[rd], kTs[rd], vas[rd]
                woT = wos[rd]
                attnoutT = pao.tile([128, 2, T], BF16, tag="ao", name=f"ao_{it}")

                def d_unit(j, tp, tsub):
                    def emit():
                        t = 4 * j + 2 * tp + tsub
                        ys = d_unit.ys
                        if tsub == 0:
                            ys = d_unit.ys = pys.tile(
                                [128, 2, C], BF16, tag="ys", name=f"ys{t}_{it}")
                        if K_DPS:
                            # pst ring recycles via exp; keeps B units from
                            # contending on pbd, and one fat ys copy
                            ypt = pst.tile([128, 2, 512], F32, tag="st",
                                           name=f"yps{t}_{it}")
                            for o2 in range(2):
                                for kc in range(2):
                                    nc.tensor.matmul(
                                        ypt[:, o2, :],
                                        attnoutT[:, kc, 128 * t:128 * (t + 1)],
                                        woT[:, kc, 512 * o2:512 * (o2 + 1)],
                                        start=(kc == 0), stop=(kc == 1))
                            nc.vector.tensor_copy(
                                ys[:, tsub, :],
                                ypt[:].rearrange("p a b -> p (a b)"))
                        else:
                            for o2 in range(2):
                                yps = pbd.tile([128, 512], F32, tag="bd",
                                               name=f"yps{t}_{o2}_{it}")
                                for kc in range(2):
                                    nc.tensor.matmul(
                                        yps[:],
                                        attnoutT[:, kc, 128 * t:128 * (t + 1)],
                                        woT[:, kc, 512 * o2:512 * (o2 + 1)],
                                        start=(kc == 0), stop=(kc == 1))
                                nc.vector.tensor_copy(
                                    ys[:, tsub, 512 * o2:512 * (o2 + 1)],
                                    yps[:])
                        if tsub == 1:
                            r0 = 512 * j + 256 * tp
                            eng = {"pool": nc.gpsimd, "act": nc.scalar,
                                   "sp": nc.sync}[K_YDMA]
                            eng.dma_start(
                                y_ap[r0:r0 + 256, :].rearrange(
                                    "(tt p) o -> p tt o", p=128),
                                ys[:])
                    return emit
                d_unit.ys = None

                for j in range(NJ):
                    blocks = [(i, bi) for i, bi in enumerate(block_info[j])
                              if bi is not None]
                    chunks = [blocks[c:c + 2] for c in range(0, len(blocks), 2)]
                    # PV bookkeeping: contributors per tq-slice
                    contrib = [[] for _ in range(4)]
                    for i, bi in blocks:
                        lo = rmin[bi[1]] if (isinstance(bi, tuple)
                                             and bi[0] == "st") else 0
                        for rp in range(lo, 4):
                            contrib[rp].append(i)
                    anat = pan.tile([128, 2, 4, 2, D], F32, tag="an",
                                    name=f"an{j}_{it}")
                    for h in range(HG):
                        m, hh = h // 2, h % 2
                        r0 = 64 * hh
                        jsl = slice(512 * j, 512 * (j + 1))
                        seen = [0] * 4
                        pvs = [None] * 4

                        def emit_pv(pt, ch, rps):
                            if "V" not in phases:
                                return
                            for c, (i, bi) in enumerate(ch):
                                lo = rmin[bi[1]] if (isinstance(bi, tuple)
                                                     and bi[0] == "st") else 0
                                for rp in rps:
                                    if rp < lo:
                                        continue
                                    seen[rp] += 1
                                    nc.tensor.matmul(
                                        pvs[rp][:, 0:65],
                                        pt[:, c, 128 * rp:128 * (rp + 1)],
                                        vaug[:, i, h, 0:65],
                                        start=(seen[rp] == 1),
                                        stop=(seen[rp] == len(contrib[rp])))

                        def norm(rp):
                            if not contrib[rp]:
                                nc.vector.memset(anat[:, m, rp, hh, :], 0.0)
                                return
                            dn = psm.tile([128, 1], F32, tag="dn")
                            nc.vector.tensor_copy(dn[:], pvs[rp][:, D:D + 1])
                            rc = psm.tile([128, 1], F32, tag="rc")
                            nc.vector.reciprocal_approx_fast(rc[:], dn[:])
                            nc.vector.tensor_scalar_mul(
                                anat[:, m, rp, hh, :], pvs[rp][:, 0:D], rc[:])

                        # pass 1: tq-slices 0,1 accumulate while chunks flow
                        if "V" in phases:
                            for rp in (0, 1):
                                if contrib[rp]:
                                    pvs[rp] = ppv.tile(
                                        [128, 512], F32, tag="pv",
                                        name=f"pv{h}_{j}_{rp}_{it}")
                        pend = deque()
                        pts = []
                        for ch in chunks:
                            nsub = len(ch)
                            # staircase blocks: cols < 128*rmin are never
                            # read downstream - trim the score matmul and exp
                            cut = [128 * rmin[bi[1]]
                                   if K_TRIM and isinstance(bi, tuple)
                                   and bi[0] == "st"
                                   else 0 for _, bi in ch]
                            st = pst.tile([128, 2, 512], F32, tag="st",
                                          name=f"st{h}_{j}_{it}")
                            for c, (i, bi) in enumerate(ch):
                                nc.tensor.matmul(
                                    st[:, c, cut[c]:],
                                    kTp[r0:r0 + 64, m, 128 * i:128 * (i + 1)],
                                    qT[r0:r0 + 64, m,
                                       512 * j + cut[c]:512 * (j + 1)],
                                    start=True, stop=True)
                            pt = ppt.tile([128, 2, 512], BF16, tag="pt")
                            # one exp per chunk when untrimmed: fixed per-op
                            # ACT cost amortizes over 1024 elements
                            if not any(cut):
                                nc.scalar.activation(
                                    pt[:, 0:nsub, :], st[:, 0:nsub, :],
                                    mybir.ActivationFunctionType.Exp,
                                    scale=SCALE)
                            else:
                                for c in range(nsub):
                                    nc.scalar.activation(
                                        pt[:, c, cut[c]:], st[:, c, cut[c]:],
                                        mybir.ActivationFunctionType.Exp,
                                        scale=SCALE)
                            for c, (i, bi) in enumerate(ch):
                                if not isinstance(bi, tuple):
                                    continue
                                if bi[0] == "st":
                                    # zero exp output where p > f - s
                                    # (cols < 128*rmin are never read)
                                    s = shifts[bi[1]]
                                    c0 = 128 * rmin[bi[1]]
                                    if use_stm:
                                        nc.vector.tensor_mul(
                                            pt[:, c, c0:], pt[:, c, c0:],
                                            stm[:, bi[1], c0:])
                                    else:
                                        nc.gpsimd.affine_select(
                                            out=pt[:, c, c0:],
                                            in_=pt[:, c, c0:],
                                            compare_op=mybir.AluOpType.is_ge,
                                            fill=0.0,
                                            base=c0 - s,
                                            pattern=[[1, 512 - c0]],
                                            channel_multiplier=-1)
                            pend.append((pt, ch))
                            pts.append((pt, ch))
                            if len(pend) > LOOKAHEAD:
                                emit_pv(*pend.popleft(), (0, 1))
                            if fillers:
                                fillers.popleft()()
                        while pend:
                            emit_pv(*pend.popleft(), (0, 1))
                        if "V" not in phases:
                            continue
                        norm(0)
                        norm(1)
                        # pass 2: tq-slices 2,3 re-sweep the kept pt tiles
                        for rp in (2, 3):
                            if contrib[rp]:
                                pvs[rp] = ppv.tile(
                                    [128, 512], F32, tag="pv",
                                    name=f"pv{h}_{j}_{rp}_{it}")
                        for pt, ch in pts:
                            emit_pv(pt, ch, (2, 3))
                        norm(2)
                        norm(3)

                        if hh == 1:
                            # both heads of pair m done: transpose natural
                            # attnout back to channel-major for out-proj
                            for half in range(2):
                                trp = ppv.tile([128, 512], F32, tag="pv",
                                               name=f"tr{m}_{half}_{j}_{it}")
                                for q2 in range(2):
                                    rp = 2 * half + q2
                                    nc.tensor.transpose(
                                        trp[:, 128 * q2:128 * (q2 + 1)],
                                        anat[:, m, rp, :, :].rearrange(
                                            "p a b -> p (a b)"),
                                        ident[:])
                                a0 = 512 * j + 256 * half
                                nc.vector.tensor_copy(
                                    attnoutT[:, m, a0:a0 + 256],
                                    trp[:, 0:256])

                    if "D" in phases and "V" in phases:
                        for tp in range(2):
                            for tsub in range(2):
                                fillers.append(d_unit(j, tp, tsub))
                while fillers:
                    fillers.popleft()()
                for u in deferred:
                    u()
                if dbg:
                    nc.sync.dma_start(dqT_ap.rearrange("p (a t) -> p a t", a=2), qT[:])
                    nc.sync.dma_start(dkT_ap.rearrange("p (a t) -> p a t", a=2), kTp[:])
                    nc.sync.dma_start(dva_ap.rearrange("p (i h e) -> p i h e", i=NT, h=HG), vaug[:])
                    nc.sync.dma_start(dao_ap.rearrange("p (a t) -> p a t", a=2), attnoutT[:])

            def prologue():
                w_ts = dma_set(0, "p")
                for u in b_units(0, w_ts, "p"):
                    u()

            prologue()
            if loop_n is None:
                body(0, 0, None)
            elif loop_n % 2:
                # odd loop count: non-pipelined fallback, single set
                with tc.For_i(0, loop_n, 1, staggered_reset=True):
                    body(0, 0, 0)
            else:
                unroll = 4 if loop_n % 4 == 0 else 2
                with tc.For_i(0, loop_n // unroll, 1, staggered_reset=True):
                    for it in range(unroll):
                        body(it, it % 2, 1 - it % 2)

    nc.compile()
    return nc


# ---------------------------------------------------------------- run harness

def _install_verbose_hook():
    install_neuronx_cc_hook()
    try:
        import libneuronxla
    except ImportError:
        return
    import traceback
    inner = bass2jax.neuronx_cc_hook

    def wrapped(*a, **kw):
        try:
            return inner(*a, **kw)
        except BaseException:
            traceback.print_exc()
            raise
    libneuronxla.neuronx_cc = wrapped


class _SpmdRunner:
    def __init__(self, nc, n_cores):
        _install_verbose_hook()
        self.nc, self.n_cores = nc, n_cores
        pname = nc.partition_id_tensor.name if nc.partition_id_tensor else None
        in_names, out_names, out_avals = [], [], []
        for alloc in nc.m.functions[0].allocations:
            if not isinstance(alloc, mybir.MemoryLocationSet):
                continue
            name = alloc.memorylocations[0].name
            if alloc.kind == "ExternalInput":
                if name != pname:
                    in_names.append(name)
            elif alloc.kind == "ExternalOutput":
                out_names.append(name)
                out_avals.append(jax.core.ShapedArray(
                    tuple(alloc.tensor_shape), mybir.dt.np(alloc.dtype)))
        self.in_names, self.out_names, self.out_avals = in_names, out_names, out_avals
        n_params = len(in_names)
        all_in = list(in_names) + list(out_names)
        if pname is not None:
            all_in.append(pname)

        def _body(*args):
            operands = list(args)
            if pname is not None:
                operands.append(partition_id_tensor())
            return tuple(_bass_exec_p.bind(
                *operands,
                out_avals=tuple(out_avals), in_names=tuple(all_in),
                out_names=tuple(out_names), lowering_input_output_aliases=(),
                sim_require_finite=True, sim_require_nnan=True, nc=nc))

        devices = jax.devices()[:n_cores]
        self.mesh = Mesh(np.asarray(devices), ("core",))
        in_specs = (PartitionSpec("core"),) * (n_params + len(out_names))
        out_specs = (PartitionSpec("core"),) * len(out_names)
        self.fn = jax.jit(shard_map(_body, mesh=self.mesh, in_specs=in_specs,
                                    out_specs=out_specs, check_rep=False),
                          keep_unused=True)
        self._shard = jax.sharding.NamedSharding(self.mesh, PartitionSpec("core"))

    def put_inputs(self, in_maps):
        arrs = []
        for name in self.in_names:
            cat = np.concatenate([np.asarray(m[name]) for m in in_maps], axis=0)
            arrs.append(jax.device_put(cat, self._shard))
        for av in self.out_avals:
            z = np.zeros((self.n_cores * av.shape[0], *av.shape[1:]), av.dtype)
            arrs.append(jax.device_put(z, self._shard))
        return arrs

    def run(self, dev_args):
        outs = self.fn(*dev_args)
        jax.block_until_ready(outs)
        return outs

    def results(self, outs):
        per_core = []
        for c in range(self.n_cores):
            per_core.append({
                name: np.asarray(outs[i]).reshape(
                    self.n_cores, *self.out_avals[i].shape)[c]
                for i, name in enumerate(self.out_names)})
        return per_core


# ---------------------------------------------------------------- host side

def _mask_blocks(mask):
    """Classify transposed 128x512 blocks of the [T,T] mask.

    Returns (block_info, uniq, shifts) where block_info[j][i] is None (all
    masked), -1 (all valid), ("st", slot) (causal staircase valid = p <=
    f - shifts[slot]), or ("mk", idx) (arbitrary pattern from uniq[idx])."""
    m2 = np.asarray(mask).reshape(T, T)
    valid = (m2 != -np.inf)          # [tq, tk]
    validT = valid.T                 # [tk, tq]
    uniq, keys = [], {}
    shifts, shift_keys = [], {}
    p_idx = np.arange(128)[:, None]
    f_idx = np.arange(512)[None, :]
    block_info = []
    for j in range(NJ):
        row = []
        for i in range(NT):
            blk = validT[128 * i:128 * (i + 1), 512 * j:512 * (j + 1)]
            if not blk.any():
                row.append(None)
                continue
            if blk.all():
                row.append(-1)
                continue
            s = 128 * i - 512 * j
            if -512 < s < 512 and np.array_equal(blk, p_idx <= f_idx - s):
                if s not in shift_keys:
                    shift_keys[s] = len(shifts)
                    shifts.append(s)
                row.append(("st", shift_keys[s]))
                continue
            k = hashlib.sha1(np.ascontiguousarray(blk)).hexdigest()
            if k not in keys:
                keys[k] = len(uniq)
                uniq.append(blk.astype(np.float32))
            row.append(("mk", keys[k]))
        block_info.append(row)
    return block_info, uniq, shifts


_CACHE = {}


def _get_runner(block_info, n_uniq, shifts=(), loop_n=None, phases="BCVD",
                cast_dma=True):
    key = (str(block_info), n_uniq, tuple(shifts), loop_n, phases,
           K_STAIR, K_YDMA, LOOKAHEAD, PT_BUFS, K_TRIM, K_DPS)
    if key not in _CACHE:
        nc = _build_nc(block_info, n_uniq, shifts=shifts, loop_n=loop_n,
                       phases=phases, cast_dma=cast_dma)
        _CACHE[key] = _SpmdRunner(nc, N_CORES)
    return _CACHE[key]


def _bf16(a):
    return np.ascontiguousarray(np.asarray(a, np.float32)).astype(
        ml_dtypes.bfloat16)


def _pack_rows(a):
    """[R*128, F] -> [128, R*F]: partition-contiguous packing for fast DMA."""
    r = a.shape[0] // 128
    return np.ascontiguousarray(
        a.reshape(r, 128, a.shape[1]).transpose(1, 0, 2).reshape(128, -1))


def _make_in_maps(x, mask, wq, wk, wv, wo):
    block_info, uniq, shifts = _mask_blocks(mask)
    x = np.asarray(x, np.float32)
    extra = {}
    if uniq:
        mk = np.stack(uniq)    # [u,128,512] -> [128, u*512]
        extra["mk"] = np.ascontiguousarray(
            mk.transpose(1, 0, 2).reshape(128, -1))
    if shifts and K_STAIR == "dve":
        p_idx = np.arange(128)[:, None]
        f_idx = np.arange(512)[None, :]
        stm = np.stack([(p_idx <= f_idx - s).astype(np.float32)
                        for s in shifts])          # [s,128,512]
        extra["stm"] = _bf16(np.ascontiguousarray(
            stm.transpose(1, 0, 2).reshape(128, -1)))
    in_maps = []
    for c in range(N_CORES):
        b, g = c // 4, c % 4
        sl = slice(HC * g, HC * (g + 1))
        in_maps.append({
            "xT": _bf16(x[b].T),
            "wqT": _pack_rows(_bf16(np.asarray(wq)[sl, :].T)),
            "wkT": _pack_rows(_bf16(np.asarray(wk)[sl, :].T)),
            "wvT": _pack_rows(_bf16(np.asarray(wv)[sl, :].T)),
            "woT": _pack_rows(_bf16(np.asarray(wo)[:, sl].T)),
            **extra,
        })
    return in_maps, block_info, len(uniq), tuple(shifts)


def kernel(x, mask, wq, wk, wv, wo):
    in_maps, block_info, n_uniq, shifts = _make_in_maps(x, mask, wq, wk, wv, wo)
    runner = _get_runner(block_info, n_uniq, shifts)
    dev = runner.put_inputs(in_maps)
    res = runner.results(runner.run(dev))
    out = np.zeros((B, T, C), np.float32)
    for c in range(N_CORES):
        out[c // 4] += res[c]["y"].astype(np.float32)
    return out


# revision 53
# speedup vs baseline: 1.0237x; 1.0050x over previous
"""Causal multi-head attention (B=2, T=2048, C=1024, H=16, D=64) on 8 trn2 cores.

Sharding: core c -> (batch c//4, head-group c%4 of 4 heads / 256 channels).
Each core computes q/k/v for its head group, causal attention, and a partial
output projection y_part[2048,1024] = attnout_g @ wo_g.T. The host sums the 4
per-group partials of each batch (the "all-reduce after wo" done host-side).

Device kernel (per core, SPMD identical program), bf16 datapath, SOFTWARE
PIPELINED across iterations: q/k/v/x/wo live in two persistent buffer sets;
iteration n runs attention (C/D) on set p while the projections (B) for
iteration n+1 are emitted as PE gap-filler units into set 1-p, interleaved
one unit per attention chunk. Phase C is exp(ACT)-paced, so the B/D filler
matmuls soak up the PE idle the in-order queue would otherwise leave.

  phase B: q^T,k^T packed 2-heads-per-128-partitions (head h in rows
           64*(h%2)..+64 of slot h//2); scores contract K=64 on those
           slices directly (no zero-padding, no scatter copies). v is
           computed NATURALLY (v[t,o] = xT-slice.T @ wvT) into
           vaug[tk, i, h, 0:64] with a ones column at 64.
  phase C: per head/tq-chunk(512): ST[tk128,tq512] = kT.T @ qT (K=64);
           P = exp(ST/8) on ScalarE -> bf16; causal staircase masked on
           P (cols >= 128*rmin only; the rest is never read). PV is
           FLIPPED: out[tq128, 65] = P-slice.T @ vaug (F=65); the 4
           tq-slice accumulation groups run as 2 passes of 2 (one PSUM
           bank per open group - a group's start clobbers other partials
           in the same bank). Column 64 is the softmax denominator,
           per-PARTITION, so normalization is an approx-recip +
           tensor_scalar_mul. attnout natural [tq, ch] is PE-transposed
           back to attnoutT[ch, t] for the out-projection.
  phase D: y = attnoutT.T @ woT per 128-row tile, emitted as filler units.

PSUM banks: scores 2x[128,2,512] (4) + PV/transpose 2x[128,512] (2) +
B/D shared ring 2x[128,512] (2).
"""
import hashlib
import numpy as np
import ml_dtypes
from collections import deque

import jax
import jax.numpy as jnp
from jax.sharding import Mesh, PartitionSpec
from jax.experimental.shard_map import shard_map

import concourse.bass as bass
import concourse.tile as tile
from concourse import bacc, mybir
from concourse import bass2jax
from concourse.bass2jax import _bass_exec_p, install_neuronx_cc_hook, partition_id_tensor

B, T, C = 2, 2048, 1024
H = 16
D = C // H            # 64
SCALE = D ** -0.5     # 0.125
N_CORES = 8
HG = H // (N_CORES // B)   # heads per core = 4
HC = HG * D                # channels per core = 256
KT = C // 128              # 8 contraction tiles
NT = T // 128              # 16 row tiles
NJ = T // 512              # 4 tq tiles
F32 = mybir.dt.float32
BF16 = mybir.dt.bfloat16

import os
K_STAIR = os.environ.get("K_STAIR", "pool")   # pool affine_select | dve mult
K_YDMA = os.environ.get("K_YDMA", "sp")       # y DMA issue queue
LOOKAHEAD = int(os.environ.get("K_LA", "2"))  # chunks in flight exp->PV
PT_BUFS = int(os.environ.get("K_PTB", "10"))  # pt pool depth (2-pass PV)
K_TRIM = os.environ.get("K_TRIM", "0") == "1"  # F-trim staircase scores/exp
K_DPS = os.environ.get("K_DPS", "0") == "1"    # y psums via pst ring (not pbd)


# ---------------------------------------------------------------- device code

def _build_nc(block_info, n_uniq, shifts=(), loop_n=None, phases="BCVD",
              cast_dma=False, dbg=False):
    """block_info[j][i] = None (skip) | -1 (full) | ("st", slot) (causal
    staircase) | ("mk", idx>=0) (general mask tile multiplied on DVE)."""
    n_shift = len(shifts)
    # per stair slot: first tq-128-slice that reads this tile
    rmin = [max(0, -(-(s - 127) // 128)) for s in shifts]
    use_stm = K_STAIR == "dve" and n_shift > 0
    nc = bacc.Bacc("TRN2", target_bir_lowering=False, debug=False,
                   num_devices=N_CORES)
    xT_ap = nc.dram_tensor("xT", [C, T], BF16, kind="ExternalInput").ap()
    # weights host-packed so each SBUF partition's bytes are contiguous in
    # DRAM (large descriptors; scattered 1KB descriptors measured ~4x slower)
    wqT_ap = nc.dram_tensor("wqT", [128, KT * HC], BF16, kind="ExternalInput").ap()
    wkT_ap = nc.dram_tensor("wkT", [128, KT * HC], BF16, kind="ExternalInput").ap()
    wvT_ap = nc.dram_tensor("wvT", [128, KT * HC], BF16, kind="ExternalInput").ap()
    woT_ap = nc.dram_tensor("woT", [128, 2 * C], BF16, kind="ExternalInput").ap()
    if n_uniq > 0:
        mk_ap = nc.dram_tensor("mk", [128, n_uniq * 512], F32,
                               kind="ExternalInput").ap()
    if use_stm:
        stm_ap = nc.dram_tensor("stm", [128, n_shift * 512], BF16,
                                kind="ExternalInput").ap()
    y_ap = nc.dram_tensor("y", [T, C], BF16, kind="ExternalOutput").ap()
    if dbg:
        dqT_ap = nc.dram_tensor("dqT", [128, 2 * T], BF16, kind="ExternalOutput").ap()
        dkT_ap = nc.dram_tensor("dkT", [128, 2 * T], BF16, kind="ExternalOutput").ap()
        dva_ap = nc.dram_tensor("dva", [128, NT * HG * 66], BF16, kind="ExternalOutput").ap()
        dao_ap = nc.dram_tensor("dao", [128, 2 * T], BF16, kind="ExternalOutput").ap()

    with tile.TileContext(nc) as tc:
        with (
            nc.allow_low_precision(reason="bf16 matmul pipeline"),
            tc.tile_pool(name="glob", bufs=1) as pg,
            tc.tile_pool(name="warm", bufs=1) as pwarm,
            tc.tile_pool(name="wp", bufs=6) as pwp,
            tc.tile_pool(name="ao", bufs=2) as pao,
            tc.tile_pool(name="an", bufs=2) as pan,
            tc.tile_pool(name="mks", bufs=2) as pmks,
            tc.tile_pool(name="pt", bufs=PT_BUFS) as ppt,
            tc.tile_pool(name="small", bufs=8) as psm,
            tc.tile_pool(name="ys", bufs=3) as pys,
            # PSUM: one open accumulation group per 2KB bank (a group's
            # start clobbers other partials sharing its bank).
            tc.tile_pool(name="pst", bufs=2, space="PSUM") as pst,   # 4 banks
            tc.tile_pool(name="ppv", bufs=2, space="PSUM") as ppv,   # 2 banks
            tc.tile_pool(name="pbd", bufs=2, space="PSUM") as pbd,   # 2 banks
        ):
            ident = pg.tile([128, 128], F32)       # PE-transpose identity
            if use_stm:
                stm = pg.tile([128, n_shift, 512], BF16)
                nc.sync.dma_start(
                    stm[:], stm_ap.rearrange("p (s f) -> p s f", f=512))

            # warm the Exp table while DMAs run
            wtile = pwarm.tile([1, 16], F32)
            nc.vector.memset(wtile[:], 0.0)
            nc.scalar.activation(wtile[:], wtile[:],
                                 mybir.ActivationFunctionType.Exp)
            import concourse.masks as _masks
            _masks.make_identity(nc, ident[:])

            # persistent double-buffered projection sets (x single-buffered:
            # its DMA for body n+1 waits body n's last projection read)
            xTs = [pg.tile([128, KT, T], BF16, name="xTs")] * 2
            qTs = [pg.tile([128, 2, T], BF16, name=f"qTs{s}") for s in range(2)]
            kTs = [pg.tile([128, 2, T], BF16, name=f"kTs{s}") for s in range(2)]
            vas = [pg.tile([128, NT, HG, 66], BF16, name=f"vas{s}") for s in range(2)]
            wos = [pg.tile([128, 2, C], BF16, name=f"wos{s}") for s in range(2)]

            def dma_set(s, it):
                """Issue input DMAs for projection set s."""
                xT = xTs[s]
                xr = xT_ap.rearrange("(k p) t -> k p t", p=128)
                w_ts = [pwp.tile([128, KT, HC], BF16, tag="w",
                                 name=f"w{qk}_{it}") for qk in range(3)]
                wqr = wqT_ap.rearrange("p (k m) -> p k m", k=KT)
                nc.sync.dma_start(w_ts[0][:, 0:KT // 2, :], wqr[:, 0:KT // 2, :])
                nc.sync.dma_start(xT[:, 0, :], xr[0])
                nc.sync.dma_start(w_ts[0][:, KT // 2:, :], wqr[:, KT // 2:, :])
                nc.sync.dma_start(xT[:, 1, :], xr[1])
                nc.sync.dma_start(w_ts[1][:], wkT_ap.rearrange("p (k m) -> p k m", k=KT))
                for kc in range(2, KT):
                    nc.sync.dma_start(xT[:, kc, :], xr[kc])
                nc.sync.dma_start(w_ts[2][:], wvT_ap.rearrange("p (k m) -> p k m", k=KT))
                nc.sync.dma_start(wos[s][:], woT_ap.rearrange("p (k m) -> p k m", k=2))
                return w_ts

            def b_units(s, w_ts, it):
                """Projection work for set s as a list of filler closures."""
                units = []
                if "B" not in phases:
                    return units
                xT, qT, kTp, vaug = xTs[s], qTs[s], kTs[s], vas[s]
                for qk in range(2):
                    dst = qT if qk == 0 else kTp
                    for m in range(2):
                        for j in range(4):
                            def u(qk=qk, m=m, j=j, dst=dst):
                                pss = pbd.tile([128, 512], F32, tag="bd",
                                               name=f"qk{qk}_{m}_{j}_{it}")
                                for kc in range(KT):
                                    nc.tensor.matmul(
                                        pss[:],
                                        w_ts[qk][:, kc, 128 * m:128 * (m + 1)],
                                        xT[:, kc, 512 * j:512 * (j + 1)],
                                        start=(kc == 0), stop=(kc == KT - 1))
                                nc.vector.tensor_copy(
                                    dst[:, m, 512 * j:512 * (j + 1)], pss[:])
                            units.append(u)
                for i in range(NT):
                    def u(i=i):
                        pvn = pbd.tile([128, 512], F32, tag="bd",
                                       name=f"vn{i}_{it}")
                        flat = pvn[:, 0:HC]
                        for kc in range(KT):
                            nc.tensor.matmul(
                                flat, xT[:, kc, 128 * i:128 * (i + 1)],
                                w_ts[2][:, kc, :],
                                start=(kc == 0), stop=(kc == KT - 1))
                        nc.vector.tensor_copy(
                            vaug[:, i, :, 0:D],
                            flat.rearrange("p (h d) -> p h d", h=HG))
                    units.append(u)
                units.append(lambda: nc.vector.memset(vaug[:, :, :, D], 1.0))
                return units

            def body(it, rd, wr):
                """Attention on set rd; fillers project into set wr."""
                fillers = deque()
                if wr is not None:
                    w_ts = dma_set(wr, it)
                if n_uniq > 0:
                    mks = pmks.tile([128, n_uniq, 512], F32, tag="mks",
                                    name=f"mks_{it}")
                    nc.sync.dma_start(mks[:], mk_ap.rearrange("p (u f) -> p u f", f=512))
                deferred = []
                if wr is not None:
                    if wr == rd:
                        # non-pipelined: projections must run after attention
                        deferred = b_units(wr, w_ts, it)
                    else:
                        fillers.extend(b_units(wr, w_ts, it))
                if "C" not in phases:
                    for u in (*fillers, *deferred):
                        u()
                    return
                qT, kTp, vaug = qTs[rd], kTs[rd], vas[rd]
                woT = wos[rd]
                attnoutT = pao.tile([128, 2, T], BF16, tag="ao", name=f"ao_{it}")

                def d_unit(j, tp, tsub):
                    def emit():
                        t = 4 * j + 2 * tp + tsub
                        ys = d_unit.ys
                        if tsub == 0:
                            ys = d_unit.ys = pys.tile(
                                [128, 2, C], BF16, tag="ys", name=f"ys{t}_{it}")
                        if K_DPS:
                            # pst ring recycles via exp; keeps B units from
                            # contending on pbd, and one fat ys copy
                            ypt = pst.tile([128, 2, 512], F32, tag="st",
                                           name=f"yps{t}_{it}")
                            for o2 in range(2):
                                for kc in range(2):
                                    nc.tensor.matmul(
                                        ypt[:, o2, :],
                                        attnoutT[:, kc, 128 * t:128 * (t + 1)],
                                        woT[:, kc, 512 * o2:512 * (o2 + 1)],
                                        start=(kc == 0), stop=(kc == 1))
                            nc.vector.tensor_copy(
                                ys[:, tsub, :],
                                ypt[:].rearrange("p a b -> p (a b)"))
                        else:
                            for o2 in range(2):
                                yps = pbd.tile([128, 512], F32, tag="bd",
                                               name=f"yps{t}_{o2}_{it}")
                                for kc in range(2):
                                    nc.tensor.matmul(
                                        yps[:],
                                        attnoutT[:, kc, 128 * t:128 * (t + 1)],
                                        woT[:, kc, 512 * o2:512 * (o2 + 1)],
                                        start=(kc == 0), stop=(kc == 1))
                                nc.vector.tensor_copy(
                                    ys[:, tsub, 512 * o2:512 * (o2 + 1)],
                                    yps[:])
                        if tsub == 1:
                            r0 = 512 * j + 256 * tp
                            eng = {"pool": nc.gpsimd, "act": nc.scalar,
                                   "sp": nc.sync}[K_YDMA]
                            eng.dma_start(
                                y_ap[r0:r0 + 256, :].rearrange(
                                    "(tt p) o -> p tt o", p=128),
                                ys[:])
                    return emit
                d_unit.ys = None

                for j in range(NJ):
                    blocks = [(i, bi) for i, bi in enumerate(block_info[j])
                              if bi is not None]
                    chunks = [blocks[c:c + 2] for c in range(0, len(blocks), 2)]
                    # PV bookkeeping: contributors per tq-slice
                    contrib = [[] for _ in range(4)]
                    for i, bi in blocks:
                        lo = rmin[bi[1]] if (isinstance(bi, tuple)
                                             and bi[0] == "st") else 0
                        for rp in range(lo, 4):
                            contrib[rp].append(i)
                    anat = pan.tile([128, 2, 4, 2, D], F32, tag="an",
                                    name=f"an{j}_{it}")
                    for h in range(HG):
                        m, hh = h // 2, h % 2
                        r0 = 64 * hh
                        jsl = slice(512 * j, 512 * (j + 1))
                        seen = [0] * 4
                        pvs = [None] * 4

                        def emit_pv(pt, ch, rps):
                            if "V" not in phases:
                                return
                            for c, (i, bi) in enumerate(ch):
                                lo = rmin[bi[1]] if (isinstance(bi, tuple)
                                                     and bi[0] == "st") else 0
                                for rp in rps:
                                    if rp < lo:
                                        continue
                                    seen[rp] += 1
                                    nc.tensor.matmul(
                                        pvs[rp][:, 0:65],
                                        pt[:, c, 128 * rp:128 * (rp + 1)],
                                        vaug[:, i, h, 0:65],
                                        start=(seen[rp] == 1),
                                        stop=(seen[rp] == len(contrib[rp])))

                        def norm(rp):
                            if not contrib[rp]:
                                nc.vector.memset(anat[:, m, rp, hh, :], 0.0)
                                return
                            dn = psm.tile([128, 1], F32, tag="dn")
                            nc.vector.tensor_copy(dn[:], pvs[rp][:, D:D + 1])
                            rc = psm.tile([128, 1], F32, tag="rc")
                            nc.vector.reciprocal_approx_fast(rc[:], dn[:])
                            nc.vector.tensor_scalar_mul(
                                anat[:, m, rp, hh, :], pvs[rp][:, 0:D], rc[:])

                        # pass 1: tq-slices 0,1 accumulate while chunks flow
                        if "V" in phases:
                            for rp in (0, 1):
                                if contrib[rp]:
                                    pvs[rp] = ppv.tile(
                                        [128, 512], F32, tag="pv",
                                        name=f"pv{h}_{j}_{rp}_{it}")
                        pend = deque()
                        pts = []
                        for ci, ch in enumerate(chunks):
                            nsub = len(ch)
                            # staircase blocks: cols < 128*rmin are never
                            # read downstream - trim the score matmul and exp
                            cut = [128 * rmin[bi[1]]
                                   if K_TRIM and isinstance(bi, tuple)
                                   and bi[0] == "st"
                                   else 0 for _, bi in ch]
                            st = pst.tile([128, 2, 512], F32, tag="st",
                                          name=f"st{h}_{j}_{it}")
                            for c, (i, bi) in enumerate(ch):
                                nc.tensor.matmul(
                                    st[:, c, cut[c]:],
                                    kTp[r0:r0 + 64, m, 128 * i:128 * (i + 1)],
                                    qT[r0:r0 + 64, m,
                                       512 * j + cut[c]:512 * (j + 1)],
                                    start=True, stop=True)
                            pt = ppt.tile([128, 2, 512], BF16, tag="pt")
                            # one exp per chunk when untrimmed: fixed per-op
                            # ACT cost amortizes over 1024 elements
                            if not any(cut):
                                nc.scalar.activation(
                                    pt[:, 0:nsub, :], st[:, 0:nsub, :],
                                    mybir.ActivationFunctionType.Exp,
                                    scale=SCALE)
                            else:
                                for c in range(nsub):
                                    nc.scalar.activation(
                                        pt[:, c, cut[c]:], st[:, c, cut[c]:],
                                        mybir.ActivationFunctionType.Exp,
                                        scale=SCALE)
                            for c, (i, bi) in enumerate(ch):
                                if not isinstance(bi, tuple):
                                    continue
                                if bi[0] == "st":
                                    # zero exp output where p > f - s
                                    # (cols < 128*rmin are never read)
                                    s = shifts[bi[1]]
                                    c0 = 128 * rmin[bi[1]]
                                    if use_stm:
                                        nc.vector.tensor_mul(
                                            pt[:, c, c0:], pt[:, c, c0:],
                                            stm[:, bi[1], c0:])
                                    else:
                                        nc.gpsimd.affine_select(
                                            out=pt[:, c, c0:],
                                            in_=pt[:, c, c0:],
                                            compare_op=mybir.AluOpType.is_ge,
                                            fill=0.0,
                                            base=c0 - s,
                                            pattern=[[1, 512 - c0]],
                                            channel_multiplier=-1)
                            pend.append((pt, ch))
                            pts.append((pt, ch))
                            if len(pend) > LOOKAHEAD:
                                emit_pv(*pend.popleft(), (0, 1))
                            # early tq-chunks drain at half rate so filler
                            # work is left for the longer exp-paced late js
                            if fillers and (j >= 2 or ci % 2 == 0):
                                fillers.popleft()()
                        while pend:
                            emit_pv(*pend.popleft(), (0, 1))
                        if "V" not in phases:
                            continue
                        norm(0)
                        norm(1)
                        # pass 2: tq-slices 2,3 re-sweep the kept pt tiles
                        for rp in (2, 3):
                            if contrib[rp]:
                                pvs[rp] = ppv.tile(
                                    [128, 512], F32, tag="pv",
                                    name=f"pv{h}_{j}_{rp}_{it}")
                        for pt, ch in pts:
                            emit_pv(pt, ch, (2, 3))
                        norm(2)
                        norm(3)

                        if hh == 1:
                            # both heads of pair m done: transpose natural
                            # attnout back to channel-major for out-proj
                            for half in range(2):
                                trp = ppv.tile([128, 512], F32, tag="pv",
                                               name=f"tr{m}_{half}_{j}_{it}")
                                for q2 in range(2):
                                    rp = 2 * half + q2
                                    nc.tensor.transpose(
                                        trp[:, 128 * q2:128 * (q2 + 1)],
                                        anat[:, m, rp, :, :].rearrange(
                                            "p a b -> p (a b)"),
                                        ident[:])
                                a0 = 512 * j + 256 * half
                                nc.vector.tensor_copy(
                                    attnoutT[:, m, a0:a0 + 256],
                                    trp[:, 0:256])

                    if "D" in phases and "V" in phases:
                        for tp in range(2):
                            for tsub in range(2):
                                fillers.append(d_unit(j, tp, tsub))
                while fillers:
                    fillers.popleft()()
                for u in deferred:
                    u()
                if dbg:
                    nc.sync.dma_start(dqT_ap.rearrange("p (a t) -> p a t", a=2), qT[:])
                    nc.sync.dma_start(dkT_ap.rearrange("p (a t) -> p a t", a=2), kTp[:])
                    nc.sync.dma_start(dva_ap.rearrange("p (i h e) -> p i h e", i=NT, h=HG), vaug[:])
                    nc.sync.dma_start(dao_ap.rearrange("p (a t) -> p a t", a=2), attnoutT[:])

            def prologue():
                w_ts = dma_set(0, "p")
                for u in b_units(0, w_ts, "p"):
                    u()

            prologue()
            if loop_n is None:
                body(0, 0, None)
            elif loop_n % 2:
                # odd loop count: non-pipelined fallback, single set
                with tc.For_i(0, loop_n, 1, staggered_reset=True):
                    body(0, 0, 0)
            else:
                unroll = 4 if loop_n % 4 == 0 else 2
                with tc.For_i(0, loop_n // unroll, 1, staggered_reset=True):
                    for it in range(unroll):
                        body(it, it % 2, 1 - it % 2)

    nc.compile()
    return nc


# ---------------------------------------------------------------- run harness

def _install_verbose_hook():
    install_neuronx_cc_hook()
    try:
        import libneuronxla
    except ImportError:
        return
    import traceback
    inner = bass2jax.neuronx_cc_hook

    def wrapped(*a, **kw):
        try:
            return inner(*a, **kw)
        except BaseException:
            traceback.print_exc()
            raise
    libneuronxla.neuronx_cc = wrapped


class _SpmdRunner:
    def __init__(self, nc, n_cores):
        _install_verbose_hook()
        self.nc, self.n_cores = nc, n_cores
        pname = nc.partition_id_tensor.name if nc.partition_id_tensor else None
        in_names, out_names, out_avals = [], [], []
        for alloc in nc.m.functions[0].allocations:
            if not isinstance(alloc, mybir.MemoryLocationSet):
                continue
            name = alloc.memorylocations[0].name
            if alloc.kind == "ExternalInput":
                if name != pname:
                    in_names.append(name)
            elif alloc.kind == "ExternalOutput":
                out_names.append(name)
                out_avals.append(jax.core.ShapedArray(
                    tuple(alloc.tensor_shape), mybir.dt.np(alloc.dtype)))
        self.in_names, self.out_names, self.out_avals = in_names, out_names, out_avals
        n_params = len(in_names)
        all_in = list(in_names) + list(out_names)
        if pname is not None:
            all_in.append(pname)

        def _body(*args):
            operands = list(args)
            if pname is not None:
                operands.append(partition_id_tensor())
            return tuple(_bass_exec_p.bind(
                *operands,
                out_avals=tuple(out_avals), in_names=tuple(all_in),
                out_names=tuple(out_names), lowering_input_output_aliases=(),
                sim_require_finite=True, sim_require_nnan=True, nc=nc))

        devices = jax.devices()[:n_cores]
        self.mesh = Mesh(np.asarray(devices), ("core",))
        in_specs = (PartitionSpec("core"),) * (n_params + len(out_names))
        out_specs = (PartitionSpec("core"),) * len(out_names)
        self.fn = jax.jit(shard_map(_body, mesh=self.mesh, in_specs=in_specs,
                                    out_specs=out_specs, check_rep=False),
                          keep_unused=True)
        self._shard = jax.sharding.NamedSharding(self.mesh, PartitionSpec("core"))

    def put_inputs(self, in_maps):
        arrs = []
        for name in self.in_names:
            cat = np.concatenate([np.asarray(m[name]) for m in in_maps], axis=0)
            arrs.append(jax.device_put(cat, self._shard))
        for av in self.out_avals:
            z = np.zeros((self.n_cores * av.shape[0], *av.shape[1:]), av.dtype)
            arrs.append(jax.device_put(z, self._shard))
        return arrs

    def run(self, dev_args):
        outs = self.fn(*dev_args)
        jax.block_until_ready(outs)
        return outs

    def results(self, outs):
        per_core = []
        for c in range(self.n_cores):
            per_core.append({
                name: np.asarray(outs[i]).reshape(
                    self.n_cores, *self.out_avals[i].shape)[c]
                for i, name in enumerate(self.out_names)})
        return per_core


# ---------------------------------------------------------------- host side

def _mask_blocks(mask):
    """Classify transposed 128x512 blocks of the [T,T] mask.

    Returns (block_info, uniq, shifts) where block_info[j][i] is None (all
    masked), -1 (all valid), ("st", slot) (causal staircase valid = p <=
    f - shifts[slot]), or ("mk", idx) (arbitrary pattern from uniq[idx])."""
    m2 = np.asarray(mask).reshape(T, T)
    valid = (m2 != -np.inf)          # [tq, tk]
    validT = valid.T                 # [tk, tq]
    uniq, keys = [], {}
    shifts, shift_keys = [], {}
    p_idx = np.arange(128)[:, None]
    f_idx = np.arange(512)[None, :]
    block_info = []
    for j in range(NJ):
        row = []
        for i in range(NT):
            blk = validT[128 * i:128 * (i + 1), 512 * j:512 * (j + 1)]
            if not blk.any():
                row.append(None)
                continue
            if blk.all():
                row.append(-1)
                continue
            s = 128 * i - 512 * j
            if -512 < s < 512 and np.array_equal(blk, p_idx <= f_idx - s):
                if s not in shift_keys:
                    shift_keys[s] = len(shifts)
                    shifts.append(s)
                row.append(("st", shift_keys[s]))
                continue
            k = hashlib.sha1(np.ascontiguousarray(blk)).hexdigest()
            if k not in keys:
                keys[k] = len(uniq)
                uniq.append(blk.astype(np.float32))
            row.append(("mk", keys[k]))
        block_info.append(row)
    return block_info, uniq, shifts


_CACHE = {}


def _get_runner(block_info, n_uniq, shifts=(), loop_n=None, phases="BCVD",
                cast_dma=True):
    key = (str(block_info), n_uniq, tuple(shifts), loop_n, phases,
           K_STAIR, K_YDMA, LOOKAHEAD, PT_BUFS, K_TRIM, K_DPS)
    if key not in _CACHE:
        nc = _build_nc(block_info, n_uniq, shifts=shifts, loop_n=loop_n,
                       phases=phases, cast_dma=cast_dma)
        _CACHE[key] = _SpmdRunner(nc, N_CORES)
    return _CACHE[key]


def _bf16(a):
    return np.ascontiguousarray(np.asarray(a, np.float32)).astype(
        ml_dtypes.bfloat16)


def _pack_rows(a):
    """[R*128, F] -> [128, R*F]: partition-contiguous packing for fast DMA."""
    r = a.shape[0] // 128
    return np.ascontiguousarray(
        a.reshape(r, 128, a.shape[1]).transpose(1, 0, 2).reshape(128, -1))


def _make_in_maps(x, mask, wq, wk, wv, wo):
    block_info, uniq, shifts = _mask_blocks(mask)
    x = np.asarray(x, np.float32)
    extra = {}
    if uniq:
        mk = np.stack(uniq)    # [u,128,512] -> [128, u*512]
        extra["mk"] = np.ascontiguousarray(
            mk.transpose(1, 0, 2).reshape(128, -1))
    if shifts and K_STAIR == "dve":
        p_idx = np.arange(128)[:, None]
        f_idx = np.arange(512)[None, :]
        stm = np.stack([(p_idx <= f_idx - s).astype(np.float32)
                        for s in shifts])          # [s,128,512]
        extra["stm"] = _bf16(np.ascontiguousarray(
            stm.transpose(1, 0, 2).reshape(128, -1)))
    in_maps = []
    for c in range(N_CORES):
        b, g = c // 4, c % 4
        sl = slice(HC * g, HC * (g + 1))
        in_maps.append({
            "xT": _bf16(x[b].T),
            "wqT": _pack_rows(_bf16(np.asarray(wq)[sl, :].T)),
            "wkT": _pack_rows(_bf16(np.asarray(wk)[sl, :].T)),
            "wvT": _pack_rows(_bf16(np.asarray(wv)[sl, :].T)),
            "woT": _pack_rows(_bf16(np.asarray(wo)[:, sl].T)),
            **extra,
        })
    return in_maps, block_info, len(uniq), tuple(shifts)


def kernel(x, mask, wq, wk, wv, wo):
    in_maps, block_info, n_uniq, shifts = _make_in_maps(x, mask, wq, wk, wv, wo)
    runner = _get_runner(block_info, n_uniq, shifts)
    dev = runner.put_inputs(in_maps)
    res = runner.results(runner.run(dev))
    out = np.zeros((B, T, C), np.float32)
    for c in range(N_CORES):
        out[c // 4] += res[c]["y"].astype(np.float32)
    return out
